# revision 30
# baseline (speedup 1.0000x reference)
"""Per-pixel dynamic 7x7 filtering (BaseTextureDiffusion._diffusion_step)
on 8 Trainium2 NeuronCores.

out[b,c,h,w] = sum_k weights[b,c,k,h,w] * pad_edge(latent)[b,c,h+i,w+j],
k = i*7+j.

Sharding: the 48 (b,c) planes are independent -> 6 planes per core.
Latent is replicate-padded on host (tiny) so the device kernel does no
edge handling.  Inputs ship fp16 (halves HBM traffic; rel err ~3e-4).

Device layout per core: partition dim = image rows (2 blocks of 128);
free dims are (col, plane) PLANE-INNERMOST, so every column shift j is
a 12j-byte offset -> always 4B-aligned -> DVE 2x fp16 mode without the
baseline's duplicated one-element-shifted latent copies.

Engine split (vs the all-DVE baseline at ~170 us/rep):
  - DVE computes ONLY the products: per row-shift i, ONE fused
    tensor_mul covers all 7 column taps via an overlapping access
    pattern (tap stride = col stride = 6 elems) -> 14 DVE ops/rep,
    measured ~3 us each (compute floor ~42 us, fully hidden).
  - The 48 adds/pixel run on the otherwise-idle TensorE: products
    accumulate into PSUM via identity-stationary matmuls (3 x N=512
    per tap, ~143 ns each; f32 psum also beats the baseline's fp16
    partial-sum precision: rel err 6.4e-4 -> 5.2e-4 incl. fp16 out).
  - Latent rows load ONCE per 128-row block (lpA + 6-row tail lpB,
    0.84 MB/rep); the 6 row-shifted copies the taps need are built
    on-chip: PE shift-matrix matmuls (eye(k=-i) on lpA accumulated with
    eye(k=128-i)[0:6] on lpB) stage each shifted tile into the 2 spare
    PSUM banks, ScalarE evicts to SBUF fp16 (exact for 0/1 weights).
    This replaces 5.6 MB/rep of shifted HBM re-reads; DMA-based
    SBUF->SBUF shifts were rejected (SWDGE fails NEFF compile inside
    For_i; HWDGE partition-shifted s2s ran at ~34 GB/s and once wedged
    the device).
  - ScalarE evicts PSUM -> SBUF fp16; DMA stores to HBM (host upcasts).
  - Weights stream as one contiguous 2.75 MB DMA per (row-block,
    row-shift) tile — 128 x 21.5 KB descriptors, bufs=5 pipeline —
    on the sync-engine HWDGE ring (measured faster than 7 per-tap
    slice DMAs once the pipeline is deep enough to hide per-DMA
    completion latency).  The scalar-engine ring carries only the
    output stores, so loads are never head-of-line blocked behind the
    PSUM-evict wait.
Measured 51.8 us/rep/core = 40.2 MB of HBM traffic (96% of it the
irreducible fp16 weights) at ~775 GB/s sustained — weights alone at
that rate are 49.7 us, so ~2 us of total overhead remains.  3.3x vs
the 169.5 us baseline, same differencing methodology (sustained
400-rep loops throttle to ~130 us/rep, so timing uses thermally
isolated 101-rep bursts).  fp8 weights rejected: quantization alone
costs ~2.3e-2 vs the 2e-2 gate, and a 1-byte operand drops DVE
tensor_tensor from 2x to 1x mode.
"""

import numpy as np

B, C, H, W = 2, 24, 256, 256
R = 7
PAD = R // 2
NCORES = 8
PLANES = B * C  # 48
PPC = PLANES // NCORES  # 6 planes per core
HP = H + 2 * PAD  # 262
WP = W + 2 * PAD  # 262
FD = PPC * W  # 1536 free elems per tap per partition
NBANK = 512  # fp32 elems per PSUM bank (matmul output limit)
DTYPE = "f16"

_cache = {}


def _split_multi_waits(nc, max_waits: int = 1):
    """walrus CoreV3 codegen in this container rejects instructions carrying
    more than one sync wait ('Too many sync wait commands').  Legalize the
    module by hoisting extra waits onto same-engine NoOps inserted directly
    before the instruction (engine stalls at the nop first — semantics
    preserved, the instruction still executes only after all conditions)."""
    import concourse.mybir as mybir

    cnt = 0
    for f in nc.m.functions:
        for b in f.blocks:
            changed = False
            new_insts = []
            for inst in b.instructions:
                si = inst.sync_info
                if si is not None and len(si.on_wait) > max_waits:
                    waits = list(si.on_wait)
                    upds = list(si.on_update)
                    chunks = [
                        waits[i : i + max_waits]
                        for i in range(0, len(waits), max_waits)
                    ]
                    for chunk in chunks[:-1]:
                        nop = mybir.InstNoOp(
                            name=f"ws_nop_{cnt}", ins=[], outs=[]
                        )
                        cnt += 1
                        nop.engine = inst.engine
                        nop.sync_info = mybir.SyncInfo(
                            on_wait=chunk, on_update=[]
                        )
                        new_insts.append(nop)
                    inst.sync_info = mybir.SyncInfo(
                        on_wait=chunks[-1], on_update=upds
                    )
                    changed = True
                new_insts.append(inst)
            if changed:
                b.instructions = new_insts
    return nc


def build_nc(
    reps: int = 1,
    loop_reps: int | None = None,
    variant: str = "full",  # "full" | "dma_only" | "w_once"
):
    """Build the per-core Bass program (SPMD; all cores run the same NEFF).

    loop_reps: if set, wrap ONE rep body in a hardware For_i loop with this
    trip count (constant NEFF size for any count; used for timing).
    """
    import concourse.bass as bass
    import concourse.mybir as mybir
    from concourse.ap import AP
    from concourse.tile import TileContext

    dt = mybir.dt.float16
    dto = mybir.dt.float32

    nc = bass.Bass("TRN2", target_bir_lowering=False, debug=False, num_devices=NCORES)
    # Host pre-transposed layouts (plane innermost):
    #   wt: [row, tap, col, plane] — one (row-block, row-shift) DMA moves 7
    #       taps = 21.5 KB per partition, fully contiguous.
    #   lp: [row, col, plane]; out: [row, col, plane].
    wt = nc.dram_tensor("wt", [H, R * R, W, PPC], dt, kind="ExternalInput").ap()
    lp = nc.dram_tensor("lp", [HP, WP, PPC], dt, kind="ExternalInput").ap()
    ident = nc.dram_tensor("ident", [128, 128], dt, kind="ExternalInput").ap()
    # Shift stationaries: shm[:, i-1] = eye(128, k=-i)  (psum[s] = lpA[s+i]);
    # tlm[:, i-1] = eye(128, k=128-i)[0:R-1]  (tail rows from lpB).
    shm = nc.dram_tensor(
        "shm", [128, R - 1, 128], dt, kind="ExternalInput"
    ).ap()
    tlm = nc.dram_tensor(
        "tlm", [R - 1, R - 1, 128], dt, kind="ExternalInput"
    ).ap()
    out = nc.dram_tensor("out", [H, W, PPC], dt, kind="ExternalOutput").ap()
    LFD = WP * PPC  # 1572 free elems per latent row tile

    with TileContext(nc) as tc:
        with tc.tile_pool(name="pool", bufs=1) as pool, tc.psum_pool(
            name="pspool", bufs=1
        ) as pspool:
            # Identity stationary for the PE accumulation matmuls; loaded
            # once, outside the timing loop.
            id_t = pool.tile([128, 128], dt, name="id_t", tag="id", bufs=1)
            nc.sync.dma_start(out=id_t[:], in_=ident)
            shm_t = pool.tile(
                [128, R - 1, 128], dt, name="shm_t", tag="shm", bufs=1
            )
            nc.sync.dma_start(out=shm_t[:], in_=shm)
            tlm_t = pool.tile(
                [R - 1, R - 1, 128], dt, name="tlm_t", tag="tlm", bufs=1
            )
            nc.sync.dma_start(out=tlm_t[:], in_=tlm)

            def rep_body(rep):
                for blk in range(H // 128):
                    r0 = blk * 128
                    # Latent rows once per block: lpA (rows r0..r0+127) and
                    # the 6-row tail lpB.  The 6 row-shifted tiles are built
                    # on-chip: PE shift-matrix matmuls stage lpA/lpB into a
                    # spare PSUM bank, ScalarE evicts to SBUF fp16
                    # (-4.8 MB HBM per rep vs re-reading shifted rows).
                    lpA = pool.tile(
                        [128, WP, PPC], dt,
                        name=f"rs_{rep}_{blk}_0", tag="rs0", bufs=2,
                    )
                    nc.sync.dma_start(out=lpA[:], in_=lp[r0 : r0 + 128])
                    lpB = pool.tile(
                        [R - 1, WP, PPC], dt,
                        name=f"lpB_{rep}_{blk}", tag="lpB", bufs=2,
                    )
                    nc.sync.dma_start(
                        out=lpB[:], in_=lp[r0 + 128 : r0 + 128 + R - 1]
                    )
                    lpA_f = lpA[:].rearrange("p a b -> p (a b)")
                    lpB_f = lpB[:].rearrange("p a b -> p (a b)")
                    rs_tiles = {0: lpA}
                    for i in range(1, R):
                        t = pool.tile(
                            [128, WP, PPC], dt,
                            name=f"rs_{rep}_{blk}_{i}", tag=f"rs{i}", bufs=2,
                        )
                        t_f = t[:].rearrange("p a b -> p (a b)")
                        for n0 in range(0, LFD, NBANK):
                            n1 = min(LFD, n0 + NBANK)
                            stg = pspool.tile(
                                [128, NBANK], dto,
                                name=f"stg_{rep}_{blk}_{i}_{n0}",
                                tag="stg", bufs=2,
                            )
                            nc.tensor.matmul(
                                stg[:, 0 : n1 - n0],
                                shm_t[:, i - 1],
                                lpA_f[:, n0:n1],
                                start=True,
                                stop=False,
                            )
                            nc.tensor.matmul(
                                stg[:, 0 : n1 - n0],
                                tlm_t[:, i - 1],
                                lpB_f[:, n0:n1],
                                start=False,
                                stop=True,
                            )
                            nc.scalar.copy(
                                out=t_f[:, n0:n1], in_=stg[:, 0 : n1 - n0]
                            )
                        rs_tiles[i] = t
                    psum_t = pspool.tile(
                        [128, FD], dto, name=f"ps_{rep}_{blk}", tag="ps", bufs=2,
                    )
                    w_once_t = None
                    wg2 = None
                    for i in range(R):
                        rs = rs_tiles[i]
                        # Two row-shifts (14 taps, 5.5 MB, 43 KB contiguous
                        # per partition) per weight DMA: fewer, larger
                        # transfers sustain the highest HBM rate; bufs=2 of
                        # double tiles keeps the same ~11 MB stream
                        # lookahead as bufs=5 singles did.
                        if variant == "w_once":
                            if w_once_t is None:
                                w_once_t = pool.tile(
                                    [128, R, W, PPC], dt,
                                    name=f"wo_{rep}_{blk}", tag="wg", bufs=2,
                                )
                                nc.sync.dma_start(
                                    out=w_once_t[:],
                                    in_=wt[r0 : r0 + 128, 0:R],
                                )
                            wg = w_once_t[:]
                        else:
                            if i % 2 == 0:
                                wg2 = pool.tile(
                                    [128, 2 * R, W, PPC], dt,
                                    name=f"wg_{rep}_{blk}_{i}", tag="wg",
                                    bufs=2,
                                )
                                ntap = min(2 * R, R * R - R * i)
                                nc.sync.dma_start(
                                    out=wg2[:, 0:ntap],
                                    in_=wt[r0 : r0 + 128, R * i : R * i + ntap],
                                )
                            off = (i % 2) * R
                            wg = wg2[:, off : off + R]
                        if variant == "dma_only":
                            continue
                        # Fused product op: prod[t, c, p] = wg[t, c, p] *
                        # lp[r+i, c+t, p].  The latent operand is an
                        # overlapping AP (tap stride == col stride == PPC);
                        # runs start at 12t bytes -> 4B-aligned -> 2x mode.
                        rsa = rs[:]
                        x = AP(
                            rsa.tensor,
                            rsa.offset,
                            [list(d) for d in rsa.ap][:1]
                            + [[PPC, R], [PPC, W], [1, PPC]],
                        )
                        prod = pool.tile(
                            [128, R * FD], dt,
                            name=f"prod_{rep}_{blk}_{i}", tag="prod", bufs=2,
                        )
                        nc.vector.tensor_mul(
                            prod[:].rearrange(
                                "p (t c pl) -> p t c pl", t=R, c=W
                            ),
                            wg[:],
                            x,
                        )
                        for t in range(R):
                            for s in range(FD // NBANK):
                                nc.tensor.matmul(
                                    psum_t[:, s * NBANK : (s + 1) * NBANK],
                                    id_t[:],
                                    prod[
                                        :,
                                        t * FD + s * NBANK : t * FD + (s + 1) * NBANK,
                                    ],
                                    start=(i == 0 and t == 0),
                                    stop=(i == R - 1 and t == R - 1),
                                )
                    # fp16 output (host upcasts): halves store traffic; adds
                    # ~4e-4 quantization, still far below the 2e-2 gate.
                    oacc = pool.tile(
                        [128, FD], dt, name=f"oacc_{rep}_{blk}", tag="oacc",
                        bufs=2,
                    )
                    if variant == "dma_only":
                        nc.vector.memset(oacc[:], 0.0)
                    else:
                        nc.scalar.copy(out=oacc[:], in_=psum_t[:])
                    nc.scalar.dma_start(
                        out=out[r0 : r0 + 128],
                        in_=oacc[:].rearrange("p (c pl) -> p c pl", pl=PPC),
                    )

            if loop_reps is not None:
                with tc.For_i(0, loop_reps, 1):
                    rep_body(0)
            else:
                for rep in range(reps):
                    rep_body(rep)
    _split_multi_waits(nc)
    return nc


def _prep_inputs(latent, weights, dtype: str = DTYPE):
    npdt = np.float16
    lat = np.asarray(latent, dtype=np.float32).reshape(PLANES, H, W)
    wts = np.asarray(weights, dtype=np.float32).reshape(PLANES, R * R, H, W)
    lpad = np.pad(lat, ((0, 0), (PAD, PAD), (PAD, PAD)), mode="edge").astype(npdt)
    eye = np.eye(128, dtype=npdt)
    # psum[s] = sum_p shm[p, s] * lpA[p] = lpA[s+i]  -> shm_i = eye(k=-i);
    # tail rows s >= 128-i come from lpB via tlm_i = eye(128, k=128-i)[0:6].
    shmat = np.ascontiguousarray(
        np.stack(
            [np.eye(128, k=-i) for i in range(1, R)], axis=1
        ).astype(npdt)
    )  # [128, 6, 128]
    tlmat = np.ascontiguousarray(
        np.stack(
            [np.eye(128, k=128 - i)[0 : R - 1] for i in range(1, R)], axis=1
        ).astype(npdt)
    )  # [6, 6, 128]
    in_maps = []
    for c in range(NCORES):
        wc = wts[c * PPC : (c + 1) * PPC]  # [6, 49, 256, 256]
        # -> [row, tap, col, plane]
        wc = np.ascontiguousarray(wc.transpose(2, 1, 3, 0).astype(npdt))
        lc = np.ascontiguousarray(
            lpad[c * PPC : (c + 1) * PPC].transpose(1, 2, 0)
        )  # [262, 262, 6]
        in_maps.append(
            {"wt": wc, "lp": lc, "ident": eye, "shm": shmat, "tlm": tlmat}
        )
    return in_maps


def _get_runner():
    """Build the Bass program and ONE sharded jit executable, cached for the
    process.  Repeated kernel() calls reuse the same loaded executable —
    creating a fresh jit per call (as run_bass_kernel_spmd does) loads a new
    executable each time and can wedge the device on the second call."""
    if "runner" in _cache:
        return _cache["runner"]

    import jax
    import concourse.mybir as mybir
    from concourse import bass2jax
    from jax.experimental.shard_map import shard_map
    from jax.sharding import Mesh, NamedSharding, PartitionSpec

    bass2jax.install_neuronx_cc_hook()
    nc = build_nc(reps=1)

    partition_name = nc.partition_id_tensor.name if nc.partition_id_tensor else None
    in_names, out_names, out_avals, zero_outs = [], [], [], []
    for alloc in nc.m.functions[0].allocations:
        if not isinstance(alloc, mybir.MemoryLocationSet):
            continue
        name = alloc.memorylocations[0].name
        if alloc.kind == "ExternalInput":
            if name != partition_name:
                in_names.append(name)
        elif alloc.kind == "ExternalOutput":
            out_names.append(name)
            shape = tuple(alloc.tensor_shape)
            dtype = mybir.dt.np(alloc.dtype)
            out_avals.append(jax.core.ShapedArray(shape, dtype))
            zero_outs.append(np.zeros(shape, dtype))
    n_params = len(in_names)
    all_in_names = list(in_names) + out_names
    if partition_name is not None:
        all_in_names.append(partition_name)

    def _body(*args):
        operands = list(args)
        if partition_name is not None:
            operands.append(bass2jax.partition_id_tensor())
        return tuple(
            bass2jax._bass_exec_p.bind(
                *operands,
                out_avals=tuple(out_avals),
                in_names=tuple(all_in_names),
                out_names=tuple(out_names),
                lowering_input_output_aliases=(),
                sim_require_finite=True,
                sim_require_nnan=True,
                nc=nc,
            )
        )

    devices = jax.devices()[:NCORES]
    mesh = Mesh(np.asarray(devices), ("core",))
    in_specs = (PartitionSpec("core"),) * (n_params + len(out_names))
    out_specs = (PartitionSpec("core"),) * len(out_names)
    sharded = jax.jit(
        shard_map(
            _body, mesh=mesh, in_specs=in_specs, out_specs=out_specs, check_rep=False
        ),
        keep_unused=True,
    )
    sh = NamedSharding(mesh, PartitionSpec("core"))
    zeros_dev = [
        jax.device_put(np.zeros((NCORES * z.shape[0], *z.shape[1:]), z.dtype), sh)
        for z in zero_outs
    ]

    def run(in_maps):
        ins_dev = [
            jax.device_put(
                np.concatenate([in_maps[c][n] for c in range(NCORES)], axis=0), sh
            )
            for n in in_names
        ]
        outs = sharded(*ins_dev, *zeros_dev)
        jax.block_until_ready(outs)
        # one output tensor: per-core [H, W, PPC] concatenated on axis 0
        return np.asarray(outs[0])

    _cache["runner"] = run
    return run


def kernel(latent, weights, window_size):
    r = int(window_size)
    assert r == R, f"kernel hardcoded for window_size={R}, got {r}"

    run = _get_runner()
    in_maps = _prep_inputs(latent, weights)
    full = run(in_maps)  # [NCORES*H, W, PPC]
    full = full.reshape(NCORES, H, W, PPC)
    full = full.transpose(0, 3, 1, 2)  # [NCORES, PPC, H, W]
    return (
        full.reshape(B, C, H, W).astype(np.float32, copy=False)
    )


# revision 31
# speedup vs baseline: 1.1088x; 1.1088x over previous
"""Per-pixel dynamic 7x7 filtering (BaseTextureDiffusion._diffusion_step)
on 8 Trainium2 NeuronCores.

out[b,c,h,w] = sum_k weights[b,c,k,h,w] * pad_edge(latent)[b,c,h+i,w+j],
k = i*7+j.

Sharding: the 48 (b,c) planes are independent -> 6 planes per core.
Latent is replicate-padded on host (tiny) so the device kernel does no
edge handling.  Inputs ship fp16 (halves HBM traffic; rel err ~3e-4).

Device layout per core: partition dim = image rows (2 blocks of 128);
free dims are (col, plane) PLANE-INNERMOST, so every column shift j is
a 12j-byte offset -> always 4B-aligned -> DVE 2x fp16 mode without the
baseline's duplicated one-element-shifted latent copies.

Engine split (vs the all-DVE baseline at ~170 us/rep):
  - DVE computes ONLY the products: per row-shift i, ONE fused
    tensor_mul covers all 7 column taps via an overlapping access
    pattern (tap stride = col stride = 6 elems) -> 14 DVE ops/rep,
    measured ~3 us each (compute floor ~42 us, fully hidden).
  - The 48 adds/pixel run on the otherwise-idle TensorE: products
    accumulate into PSUM via identity-stationary matmuls (3 x N=512
    per tap, ~143 ns each; f32 psum also beats the baseline's fp16
    partial-sum precision: rel err 6.4e-4 -> 5.2e-4 incl. fp16 out).
  - Latent rows load ONCE per 128-row block (lpA + 6-row tail lpB,
    0.84 MB/rep); the 6 row-shifted copies the taps need are built
    on-chip: PE shift-matrix matmuls (eye(k=-i) on lpA accumulated with
    eye(k=128-i)[0:6] on lpB) stage each shifted tile into the 2 spare
    PSUM banks, ScalarE evicts to SBUF fp16 (exact for 0/1 weights).
    This replaces 5.6 MB/rep of shifted HBM re-reads; DMA-based
    SBUF->SBUF shifts were rejected (SWDGE fails NEFF compile inside
    For_i; HWDGE partition-shifted s2s ran at ~34 GB/s and once wedged
    the device).
  - ScalarE evicts PSUM -> SBUF fp16; DMA stores to HBM (host upcasts).
  - Weights stream as one contiguous 2.75 MB DMA per (row-block,
    row-shift) tile — 128 x 21.5 KB descriptors, bufs=5 pipeline —
    on the sync-engine HWDGE ring (measured faster than 7 per-tap
    slice DMAs once the pipeline is deep enough to hide per-DMA
    completion latency).  The scalar-engine ring carries only the
    output stores, so loads are never head-of-line blocked behind the
    PSUM-evict wait.
Measured 51.8 us/rep/core = 40.2 MB of HBM traffic (96% of it the
irreducible fp16 weights) at ~775 GB/s sustained — weights alone at
that rate are 49.7 us, so ~2 us of total overhead remains.  3.3x vs
the 169.5 us baseline, same differencing methodology (sustained
400-rep loops throttle to ~130 us/rep, so timing uses thermally
isolated 101-rep bursts).  fp8 weights rejected: quantization alone
costs ~2.3e-2 vs the 2e-2 gate, and a 1-byte operand drops DVE
tensor_tensor from 2x to 1x mode.
"""

import numpy as np

B, C, H, W = 2, 24, 256, 256
R = 7
PAD = R // 2
NCORES = 8
PLANES = B * C  # 48
PPC = PLANES // NCORES  # 6 planes per core
HP = H + 2 * PAD  # 262
WP = W + 2 * PAD  # 262
FD = PPC * W  # 1536 free elems per tap per partition
NBANK = 512  # fp32 elems per PSUM bank (matmul output limit)
DTYPE = "f16"

_cache = {}


def _split_multi_waits(nc, max_waits: int = 1):
    """walrus CoreV3 codegen in this container rejects instructions carrying
    more than one sync wait ('Too many sync wait commands').  Legalize the
    module by hoisting extra waits onto same-engine NoOps inserted directly
    before the instruction (engine stalls at the nop first — semantics
    preserved, the instruction still executes only after all conditions)."""
    import concourse.mybir as mybir

    cnt = 0
    for f in nc.m.functions:
        for b in f.blocks:
            changed = False
            new_insts = []
            for inst in b.instructions:
                si = inst.sync_info
                if si is not None and len(si.on_wait) > max_waits:
                    waits = list(si.on_wait)
                    upds = list(si.on_update)
                    chunks = [
                        waits[i : i + max_waits]
                        for i in range(0, len(waits), max_waits)
                    ]
                    for chunk in chunks[:-1]:
                        nop = mybir.InstNoOp(
                            name=f"ws_nop_{cnt}", ins=[], outs=[]
                        )
                        cnt += 1
                        nop.engine = inst.engine
                        nop.sync_info = mybir.SyncInfo(
                            on_wait=chunk, on_update=[]
                        )
                        new_insts.append(nop)
                    inst.sync_info = mybir.SyncInfo(
                        on_wait=chunks[-1], on_update=upds
                    )
                    changed = True
                new_insts.append(inst)
            if changed:
                b.instructions = new_insts
    return nc


def build_nc(
    reps: int = 1,
    loop_reps: int | None = None,
    variant: str = "full",  # "full" | "dma_only" | "w_once"
):
    """Build the per-core Bass program (SPMD; all cores run the same NEFF).

    loop_reps: if set, wrap ONE rep body in a hardware For_i loop with this
    trip count (constant NEFF size for any count; used for timing).
    """
    import concourse.bass as bass
    import concourse.mybir as mybir
    from concourse.ap import AP
    from concourse.tile import TileContext

    dt = mybir.dt.float16
    dto = mybir.dt.float32

    nc = bass.Bass("TRN2", target_bir_lowering=False, debug=False, num_devices=NCORES)
    # Host pre-transposed layouts (plane innermost):
    #   wt: [row, tap, col, plane] — one (row-block, row-shift) DMA moves 7
    #       taps = 21.5 KB per partition, fully contiguous.
    #   lp: [row, col, plane]; out: [row, col, plane].
    wt = nc.dram_tensor("wt", [H, R * R, W, PPC], dt, kind="ExternalInput").ap()
    lp = nc.dram_tensor("lp", [HP, WP, PPC], dt, kind="ExternalInput").ap()
    ident = nc.dram_tensor("ident", [128, 128], dt, kind="ExternalInput").ap()
    # Shift stationaries: shm[:, i-1] = eye(128, k=-i)  (psum[s] = lpA[s+i]);
    # tlm[:, i-1] = eye(128, k=128-i)[0:R-1]  (tail rows from lpB).
    shm = nc.dram_tensor(
        "shm", [128, R - 1, 128], dt, kind="ExternalInput"
    ).ap()
    tlm = nc.dram_tensor(
        "tlm", [R - 1, R - 1, 128], dt, kind="ExternalInput"
    ).ap()
    out = nc.dram_tensor("out", [H, W, PPC], dt, kind="ExternalOutput").ap()
    LFD = WP * PPC  # 1572 free elems per latent row tile

    with TileContext(nc) as tc:
        with tc.tile_pool(name="pool", bufs=1) as pool, tc.psum_pool(
            name="pspool", bufs=1
        ) as pspool:
            # Identity stationary for the PE accumulation matmuls; loaded
            # once, outside the timing loop.
            id_t = pool.tile([128, 128], dt, name="id_t", tag="id", bufs=1)
            nc.sync.dma_start(out=id_t[:], in_=ident)
            shm_t = pool.tile(
                [128, R - 1, 128], dt, name="shm_t", tag="shm", bufs=1
            )
            nc.sync.dma_start(out=shm_t[:], in_=shm)
            tlm_t = pool.tile(
                [R - 1, R - 1, 128], dt, name="tlm_t", tag="tlm", bufs=1
            )
            nc.sync.dma_start(out=tlm_t[:], in_=tlm)

            def rep_body(rep):
                for blk in range(H // 128):
                    r0 = blk * 128
                    # Latent rows once per block: lpA (rows r0..r0+127) and
                    # the 6-row tail lpB.  The 6 row-shifted tiles are built
                    # on-chip: PE shift-matrix matmuls stage lpA/lpB into a
                    # spare PSUM bank, ScalarE evicts to SBUF fp16
                    # (-4.8 MB HBM per rep vs re-reading shifted rows).
                    lpA = pool.tile(
                        [128, WP, PPC], dt,
                        name=f"rs_{rep}_{blk}_0", tag="rs0", bufs=2,
                    )
                    nc.sync.dma_start(out=lpA[:], in_=lp[r0 : r0 + 128])
                    lpB = pool.tile(
                        [R - 1, WP, PPC], dt,
                        name=f"lpB_{rep}_{blk}", tag="lpB", bufs=2,
                    )
                    nc.sync.dma_start(
                        out=lpB[:], in_=lp[r0 + 128 : r0 + 128 + R - 1]
                    )
                    lpA_f = lpA[:].rearrange("p a b -> p (a b)")
                    lpB_f = lpB[:].rearrange("p a b -> p (a b)")
                    rs_tiles = {0: lpA}
                    for i in range(1, R):
                        t = pool.tile(
                            [128, WP, PPC], dt,
                            name=f"rs_{rep}_{blk}_{i}", tag=f"rs{i}", bufs=2,
                        )
                        t_f = t[:].rearrange("p a b -> p (a b)")
                        for n0 in range(0, LFD, NBANK):
                            n1 = min(LFD, n0 + NBANK)
                            stg = pspool.tile(
                                [128, NBANK], dto,
                                name=f"stg_{rep}_{blk}_{i}_{n0}",
                                tag="stg", bufs=2,
                            )
                            nc.tensor.matmul(
                                stg[:, 0 : n1 - n0],
                                shm_t[:, i - 1],
                                lpA_f[:, n0:n1],
                                start=True,
                                stop=False,
                            )
                            nc.tensor.matmul(
                                stg[:, 0 : n1 - n0],
                                tlm_t[:, i - 1],
                                lpB_f[:, n0:n1],
                                start=False,
                                stop=True,
                            )
                            nc.scalar.copy(
                                out=t_f[:, n0:n1], in_=stg[:, 0 : n1 - n0]
                            )
                        rs_tiles[i] = t
                    psum_t = pspool.tile(
                        [128, FD], dto, name=f"ps_{rep}_{blk}", tag="ps", bufs=2,
                    )
                    w_once_t = None
                    for i in range(R):
                        rs = rs_tiles[i]
                        # 7 taps of row-shift i: per-tap slice DMAs (finer
                        # completion granularity pipelines better than one
                        # 2.75 MB transfer).
                        if variant == "w_once":
                            if w_once_t is None:
                                w_once_t = pool.tile(
                                    [128, R, W, PPC], dt,
                                    name=f"wo_{rep}_{blk}", tag="wg", bufs=2,
                                )
                                nc.sync.dma_start(
                                    out=w_once_t[:],
                                    in_=wt[r0 : r0 + 128, 0:R],
                                )
                            wg = w_once_t
                        else:
                            wg = pool.tile(
                                [128, R, W, PPC], dt,
                                name=f"wg_{rep}_{blk}_{i}", tag="wg", bufs=5,
                            )
                            nc.sync.dma_start(
                                out=wg[:],
                                in_=wt[r0 : r0 + 128, R * i : R * i + R],
                            )
                        if variant == "dma_only":
                            continue
                        # Fused product op: prod[t, c, p] = wg[t, c, p] *
                        # lp[r+i, c+t, p].  The latent operand is an
                        # overlapping AP (tap stride == col stride == PPC);
                        # runs start at 12t bytes -> 4B-aligned -> 2x mode.
                        rsa = rs[:]
                        x = AP(
                            rsa.tensor,
                            rsa.offset,
                            [list(d) for d in rsa.ap][:1]
                            + [[PPC, R], [PPC, W], [1, PPC]],
                        )
                        prod = pool.tile(
                            [128, R * FD], dt,
                            name=f"prod_{rep}_{blk}_{i}", tag="prod", bufs=2,
                        )
                        nc.vector.tensor_mul(
                            prod[:].rearrange(
                                "p (t c pl) -> p t c pl", t=R, c=W
                            ),
                            wg[:],
                            x,
                        )
                        for t in range(R):
                            for s in range(FD // NBANK):
                                nc.tensor.matmul(
                                    psum_t[:, s * NBANK : (s + 1) * NBANK],
                                    id_t[:],
                                    prod[
                                        :,
                                        t * FD + s * NBANK : t * FD + (s + 1) * NBANK,
                                    ],
                                    start=(i == 0 and t == 0),
                                    stop=(i == R - 1 and t == R - 1),
                                )
                    # fp16 output (host upcasts): halves store traffic; adds
                    # ~4e-4 quantization, still far below the 2e-2 gate.
                    oacc = pool.tile(
                        [128, FD], dt, name=f"oacc_{rep}_{blk}", tag="oacc",
                        bufs=2,
                    )
                    if variant == "dma_only":
                        nc.vector.memset(oacc[:], 0.0)
                    else:
                        nc.scalar.copy(out=oacc[:], in_=psum_t[:])
                    nc.scalar.dma_start(
                        out=out[r0 : r0 + 128],
                        in_=oacc[:].rearrange("p (c pl) -> p c pl", pl=PPC),
                    )

            if loop_reps is not None:
                with tc.For_i(0, loop_reps, 1):
                    rep_body(0)
            else:
                for rep in range(reps):
                    rep_body(rep)
    _split_multi_waits(nc)
    return nc


def _prep_inputs(latent, weights, dtype: str = DTYPE):
    npdt = np.float16
    lat = np.asarray(latent, dtype=np.float32).reshape(PLANES, H, W)
    wts = np.asarray(weights, dtype=np.float32).reshape(PLANES, R * R, H, W)
    lpad = np.pad(lat, ((0, 0), (PAD, PAD), (PAD, PAD)), mode="edge").astype(npdt)
    eye = np.eye(128, dtype=npdt)
    # psum[s] = sum_p shm[p, s] * lpA[p] = lpA[s+i]  -> shm_i = eye(k=-i);
    # tail rows s >= 128-i come from lpB via tlm_i = eye(128, k=128-i)[0:6].
    shmat = np.ascontiguousarray(
        np.stack(
            [np.eye(128, k=-i) for i in range(1, R)], axis=1
        ).astype(npdt)
    )  # [128, 6, 128]
    tlmat = np.ascontiguousarray(
        np.stack(
            [np.eye(128, k=128 - i)[0 : R - 1] for i in range(1, R)], axis=1
        ).astype(npdt)
    )  # [6, 6, 128]
    in_maps = []
    for c in range(NCORES):
        wc = wts[c * PPC : (c + 1) * PPC]  # [6, 49, 256, 256]
        # -> [row, tap, col, plane]
        wc = np.ascontiguousarray(wc.transpose(2, 1, 3, 0).astype(npdt))
        lc = np.ascontiguousarray(
            lpad[c * PPC : (c + 1) * PPC].transpose(1, 2, 0)
        )  # [262, 262, 6]
        in_maps.append(
            {"wt": wc, "lp": lc, "ident": eye, "shm": shmat, "tlm": tlmat}
        )
    return in_maps


def _get_runner():
    """Build the Bass program and ONE sharded jit executable, cached for the
    process.  Repeated kernel() calls reuse the same loaded executable —
    creating a fresh jit per call (as run_bass_kernel_spmd does) loads a new
    executable each time and can wedge the device on the second call."""
    if "runner" in _cache:
        return _cache["runner"]

    import jax
    import concourse.mybir as mybir
    from concourse import bass2jax
    from jax.experimental.shard_map import shard_map
    from jax.sharding import Mesh, NamedSharding, PartitionSpec

    bass2jax.install_neuronx_cc_hook()
    nc = build_nc(reps=1)

    partition_name = nc.partition_id_tensor.name if nc.partition_id_tensor else None
    in_names, out_names, out_avals, zero_outs = [], [], [], []
    for alloc in nc.m.functions[0].allocations:
        if not isinstance(alloc, mybir.MemoryLocationSet):
            continue
        name = alloc.memorylocations[0].name
        if alloc.kind == "ExternalInput":
            if name != partition_name:
                in_names.append(name)
        elif alloc.kind == "ExternalOutput":
            out_names.append(name)
            shape = tuple(alloc.tensor_shape)
            dtype = mybir.dt.np(alloc.dtype)
            out_avals.append(jax.core.ShapedArray(shape, dtype))
            zero_outs.append(np.zeros(shape, dtype))
    n_params = len(in_names)
    all_in_names = list(in_names) + out_names
    if partition_name is not None:
        all_in_names.append(partition_name)

    def _body(*args):
        operands = list(args)
        if partition_name is not None:
            operands.append(bass2jax.partition_id_tensor())
        return tuple(
            bass2jax._bass_exec_p.bind(
                *operands,
                out_avals=tuple(out_avals),
                in_names=tuple(all_in_names),
                out_names=tuple(out_names),
                lowering_input_output_aliases=(),
                sim_require_finite=True,
                sim_require_nnan=True,
                nc=nc,
            )
        )

    devices = jax.devices()[:NCORES]
    mesh = Mesh(np.asarray(devices), ("core",))
    in_specs = (PartitionSpec("core"),) * (n_params + len(out_names))
    out_specs = (PartitionSpec("core"),) * len(out_names)
    sharded = jax.jit(
        shard_map(
            _body, mesh=mesh, in_specs=in_specs, out_specs=out_specs, check_rep=False
        ),
        keep_unused=True,
    )
    sh = NamedSharding(mesh, PartitionSpec("core"))
    zeros_dev = [
        jax.device_put(np.zeros((NCORES * z.shape[0], *z.shape[1:]), z.dtype), sh)
        for z in zero_outs
    ]

    def run(in_maps):
        ins_dev = [
            jax.device_put(
                np.concatenate([in_maps[c][n] for c in range(NCORES)], axis=0), sh
            )
            for n in in_names
        ]
        outs = sharded(*ins_dev, *zeros_dev)
        jax.block_until_ready(outs)
        # one output tensor: per-core [H, W, PPC] concatenated on axis 0
        return np.asarray(outs[0])

    _cache["runner"] = run
    return run


def kernel(latent, weights, window_size):
    r = int(window_size)
    assert r == R, f"kernel hardcoded for window_size={R}, got {r}"

    run = _get_runner()
    in_maps = _prep_inputs(latent, weights)
    full = run(in_maps)  # [NCORES*H, W, PPC]
    full = full.reshape(NCORES, H, W, PPC)
    full = full.transpose(0, 3, 1, 2)  # [NCORES, PPC, H, W]
    return (
        full.reshape(B, C, H, W).astype(np.float32, copy=False)
    )
